# revision 15
# baseline (speedup 1.0000x reference)
"""Trainium2 Bass kernel for BinaryLinearUnit:
    y = sign(x) @ sign(w).T ; BatchNorm1d(train) ; * gamma + beta

Strategy: data-parallel over the batch dim across 8 NeuronCores, with
the weight matrix distributed as FP8 sign values through AllGathers
instead of every core reading the full fp32 w from HBM (64MB/core in
the f32-replicated version; that kernel was HBM-bound).  Per core:

  - read only the core's own 1/8 slice of w in fp32 (8MB), sign it to
    fp8 on ACT, and distribute it with FOUR slot AllGathers (one per
    owned output tile, 0.5MB in / 4MB out each) that pipeline through
    the collective queue; a single 16MB gather serializes behind the
    ~25us collective latency floor and its transport, stalling the PE.
  - remote output tiles are processed slot-major (4,8,..,28 then
    5,9,..,29, ...) so tile groups become ready in collective order.
  - a small fp32 "head" (output tiles 0..3, replicated to all cores)
    is signed locally so the PE has matmul work while the collectives
    are in flight; ot 0-1 head matmuls are emitted kp-interleaved so
    8 psum chains trickle along with the arriving x stream.
  - x signs are first-class on the ACT queue: head/slice signs are
    spliced between x-tile signs only at points where their own loads
    have certainly landed, so the in-order ACT queue never stalls the
    x->sign->matmul chain on the w path.
  - y.T is computed with fp8 DoubleRow matmuls (sign values exact in
    fp8e4m3, PSUM accumulates fp32; y is an even integer <= 4096,
    exact in f16).
  - sync-BN: per-core [mean/8, E[y^2]/8] AllGathers in 5 groups of
    the processing order sized [7,7,7,7,4] tiles, so the 4th group's
    collective completes before the last matmul and only the final
    small collective plus a short normalize remains in the tail
    (group 3's normalize fills the final collective's latency).
  - normalized output is stored as f16 and widened to f32 on the
    host, halving store traffic.

Indexing: all standing per-tile state (mvT/yTt/scal/nbias/gbt) is
addressed by PROCESSING POSITION; only the yt store offset and the
gathered-w source use the real channel tile.  The host packs gamma/
beta in processing order and un-permutes nothing (yt rows are stored
at their real offsets).

Host side only reshapes/transposes/casts.
"""

import numpy as np

import concourse.bass as bass
import concourse.mybir as mybir
import concourse.tile as tile
from concourse import bacc
from concourse.bass import ts
from concourse.bass_utils import run_bass_kernel_spmd
from concourse.tile_rust import add_dep_helper

N_CORES = 8
BN_EPS = 1e-5
HEAD = 6                      # leading output tiles replicated in f32

f32 = mybir.dt.float32
f16 = mybir.dt.float16
fp8 = mybir.dt.float8e4


def proc_order(OT, n_cores=N_CORES, head=HEAD):
    """head tiles in order, then remote tiles slot-major."""
    OSL = OT // n_cores
    order = list(range(head))
    for j in range(OSL):
        for r in range(n_cores):
            ot = r * OSL + j
            if ot >= head or r > 0:
                if ot not in order:
                    order.append(ot)
    assert len(order) == OT and sorted(order) == list(range(OT))
    return order


def build(B, IN, OUT, n_cores=N_CORES):
    """Build the per-core SPMD module. Shapes: x [B, IN], w [OUT, IN]."""
    Bc = B // n_cores          # batch rows per core
    KT = IN // 128             # k tiles (contraction)
    OT = OUT // 128            # output-feature tiles
    NB = min(512, Bc)          # matmul free dim / psum bank width
    BT = Bc // NB              # b tiles per core
    KP = KT // 2               # DoubleRow k steps
    OSL = OT // n_cores        # output tiles owned per core
    order = proc_order(OT, n_cores, HEAD)

    # BN stats groups over processing positions.
    if OT == 32:
        GS = [7, 7, 7, 6, 5]
    else:
        GS = [OT - OT // 2, OT // 2]
    NSPLIT = len(GS)
    GO = [sum(GS[:q]) for q in range(NSPLIT)]
    group_end = {GO[q] + GS[q] - 1: q for q in range(NSPLIT)}

    nc = bacc.Bacc("TRN2", target_bir_lowering=False, debug=False,
                   num_devices=n_cores)

    # Per-core external I/O (host pre-transposed, K-major):
    #   xt[k, b] = x[core*Bc + b, k]
    #   whead[j, p, ks, o] = w[j*128 + o, ks*128 + p]          (ots 0..HEAD-1)
    #   wsl[j, p, ks, o]   = w[(core*OSL+j)*128 + o, ks*128+p] (own slice)
    #   gb[p, 0, pos] = gamma[order[pos]*128+p] (processing order)
    #   yt[o, b] = out[core*Bc + b, o]  (f16, widened on host)
    xt = nc.dram_tensor("xt", [IN, Bc], f32, kind="ExternalInput")
    whead = nc.dram_tensor("whead", [HEAD, 128, KT, 128], f32,
                           kind="ExternalInput")
    wsl = nc.dram_tensor("wsl", [OSL, 128, KT, 128], f32,
                         kind="ExternalInput")
    gb = nc.dram_tensor("gb", [128, 2, OT], f32, kind="ExternalInput")
    yt = nc.dram_tensor("yt", [OUT, Bc], f16, kind="ExternalOutput")

    # w sign-sharing collective buffers (fp8), one gather per slot.
    wcin = nc.dram_tensor("wcin", [OSL, 128, KT, 128], fp8)
    wcout = [
        nc.dram_tensor(f"wcout{j}", [n_cores, 128, KT, 128], fp8,
                       addr_space="Shared")
        for j in range(OSL)
    ]

    # Stats collective bounce buffers per group: [mean/8, E[y^2]/8].
    ccin = [
        nc.dram_tensor(f"ccin{q}", [128, 2 * GS[q]], f32) for q in range(NSPLIT)
    ]
    ccout = [
        nc.dram_tensor(
            f"ccout{q}", [n_cores * 128, 2 * GS[q]], f32, addr_space="Shared"
        )
        for q in range(NSPLIT)
    ]

    with tile.TileContext(nc) as tc:
        with (
            tc.tile_pool(name="big", bufs=1) as big,
            tc.tile_pool(name="xs", bufs=3) as xsp,
            tc.tile_pool(name="ws", bufs=2) as wsp,
            tc.tile_pool(name="wg", bufs=3) as wgp,
            tc.tile_pool(name="sw", bufs=6) as swp,
            tc.tile_pool(name="wc", bufs=4) as wcp,
            tc.tile_pool(name="ps", bufs=4, space="PSUM") as psp,
            tc.tile_pool(name="st", bufs=2) as stp,
            tc.tile_pool(name="outp", bufs=4) as outp,
        ):
            # Standing tensors (pos-indexed except where noted)
            sxT = big.tile([128, KT, Bc], fp8)          # sign(x).T, K-major
            yTt = big.tile([128, OT, Bc], f16)          # y.T
            mvT = big.tile([128, 2, OT], f32)           # per-core [mean, var]
            gbt = big.tile([128, 2, OT], f32)           # [gamma; beta]
            scal = big.tile([128, OT], f32)             # gamma * rstd
            nbias = big.tile([128, OT], f32)            # beta - mean * scal
            grTs = [None] * NSPLIT                      # global stats tiles

            hk = KT // 2

            # --- head path: ot 0-1 fp32 loads go first on the ACT queue
            # (their signs are spliced into the x stream); ots 2..HEAD-1
            # are loaded+signed after the x loop so the collective input
            # stores are not queued behind them.
            head_tiles = {}     # j -> (swt_fp8, [(dst_slice, src_tile), ...])

            def head_load(j):
                swt = swp.tile([128, KT, 128], fp8, tag="swt", name="swt")
                halves = []
                for h in range(2):
                    wst = wsp.tile([128, hk, 128], f32, tag="wst", name="wst")
                    nc.scalar.dma_start(
                        out=wst[:], in_=whead[j, :, h * hk : (h + 1) * hk, :]
                    )
                    halves.append((swt[:, h * hk : (h + 1) * hk, :], wst))
                head_tiles[j] = (swt, halves)

            head_load(0)
            head_load(1)

            # --- own-slice path (fp32 loads queue behind the head on ACT)
            wcs_tiles = []
            wsl_halves = []
            for j in range(OSL):
                wcs = wcp.tile([128, KT, 128], fp8, tag="wcs", name="wcs")
                for h in range(2):
                    wst = wgp.tile([128, hk, 128], f32, tag="wsl", name="wsl")
                    nc.scalar.dma_start(
                        out=wst[:], in_=wsl[j, :, h * hk : (h + 1) * hk, :]
                    )
                    wsl_halves.append((wcs[:, h * hk : (h + 1) * hk, :], wst))
                wcs_tiles.append(wcs)

            # --- x sign stream: x signs are first-class; head/slice signs
            # and wcin stores + slot collectives are spliced at points
            # where their loads/signs have certainly completed.
            head_sign_after = {2: 0, 3: 1, 6: 2, 8: 3}
            wsl_sign_after = {10: 0, 12: 1, 14: 2, 16: 3, 18: 4, 20: 5,
                              22: 6, 24: 7}
            wcin_after = {13: 0, 17: 1, 21: 2, 25: 3}

            def splice(ks):
                h = head_sign_after.get(ks)
                if h is not None:
                    dst, src = head_tiles[h // 2][1][h % 2]
                    nc.scalar.sign(dst, src[:])
                h = wsl_sign_after.get(ks)
                if h is not None:
                    dst, src = wsl_halves[h]
                    nc.scalar.sign(dst, src[:])
                j = wcin_after.get(ks)
                if j is not None:
                    nc.scalar.dma_start(out=wcin[j], in_=wcs_tiles[j][:])
                    nc.gpsimd.collective_compute(
                        "AllGather",
                        mybir.AluOpType.bypass,
                        replica_groups=[list(range(n_cores))],
                        ins=[wcin[j]],
                        outs=[wcout[j][:]],
                    )

            for ks in range(KT):
                xst = xsp.tile([128, Bc], f32, tag="xst", name="xst")
                nc.sync.dma_start(out=xst[:], in_=xt[ts(ks, 128), :])
                nc.scalar.sign(sxT[:, ks, :], xst[:])
                splice(ks)

            nc.sync.dma_start(out=gbt[:], in_=gb[:])

            # remaining head tiles: loads queue on ACT behind the wcin
            # stores; signs pace themselves behind the loads.
            for j in range(2, HEAD):
                head_load(j)
                for dst, src_t in head_tiles[j][1]:
                    nc.scalar.sign(dst, src_t[:])

            def w_load_fp8(ot):
                swt = swp.tile([128, KT, 128], fp8, tag="swt", name="swt")
                nc.sync.dma_start(out=swt[:], in_=wcout[ot % OSL][ot // OSL])
                return swt

            def mm_tile(pos, swt, interleave_with=None):
                """Matmuls for one (or two kp-interleaved) positions."""
                tiles = [(pos, swt)]
                if interleave_with is not None:
                    tiles.append(interleave_with)
                psums = {
                    (p, bt): psp.tile([128, NB], f32, tag=f"ps{bt}",
                                      name=f"psum{bt}")
                    for p, _ in tiles
                    for bt in range(BT)
                }
                for kp in range(KP):
                    for p, sw in tiles:
                        for bt in range(BT):
                            nc.tensor.matmul(
                                psums[p, bt][:],
                                lhsT=sw[:, 2 * kp : 2 * kp + 2, :],
                                rhs=sxT[:, 2 * kp : 2 * kp + 2, ts(bt, NB)],
                                start=(kp == 0),
                                stop=(kp == KP - 1),
                                perf_mode=mybir.MatmulPerfMode.DoubleRow,
                            )
                aggr = None
                for p, _ in tiles:
                    st6 = stp.tile([128, BT, 6], f32, tag="st6", name="st6",
                                   bufs=4)
                    for bt in range(BT):
                        nc.vector.bn_stats(st6[:, bt, :], psums[p, bt][:])
                        nc.vector.tensor_copy(yTt[:, p, ts(bt, NB)],
                                              psums[p, bt][:])
                    aggr = nc.vector.bn_aggr(mvT[:, :, p], st6[:])
                return aggr

            def stats_pre(q):
                o0, HOT = GO[q], GS[q]
                osl = slice(o0, o0 + HOT)
                arT = stp.tile([128, 2, HOT], f32, tag="arT", name="arT")
                tmp = stp.tile([128, HOT], f32, tag="tmp_ar", name="tmp_ar")
                nc.vector.tensor_scalar_mul(arT[:, 0, :], mvT[:, 0, osl],
                                            1.0 / n_cores)
                nc.vector.tensor_mul(tmp[:], mvT[:, 0, osl], mvT[:, 0, osl])
                nc.vector.tensor_add(tmp[:], tmp[:], mvT[:, 1, osl])
                nc.vector.tensor_scalar_mul(arT[:, 1, :], tmp[:], 1.0 / n_cores)
                nc.sync.dma_start(out=ccin[q][:], in_=arT[:])
                nc.gpsimd.collective_compute(
                    "AllGather",
                    mybir.AluOpType.bypass,
                    replica_groups=[list(range(n_cores))],
                    ins=[ccin[q][:]],
                    outs=[ccout[q][:]],
                )
                grA = big.tile([128, n_cores, 2, HOT], f32, name=f"grA{q}")  # noqa
                nc.gpsimd.dma_start(
                    out=grA[:],
                    in_=ccout[q][:].rearrange("(r p) j -> p r j", p=128),
                )
                grTs[q] = grA

            def stats_post(q, anchor=None):
                o0, HOT = GO[q], GS[q]
                osl = slice(o0, o0 + HOT)
                grA = grTs[q]
                grT = stp.tile([128, 2, HOT], f32, tag="grT", name="grT")
                first = nc.vector.tensor_reduce(
                    grT[:],
                    grA[:].rearrange("p r two h -> p two h r"),
                    axis=mybir.AxisListType.X,
                    op=mybir.AluOpType.add,
                )
                if anchor is not None:
                    add_dep_helper(first.ins, anchor.ins, sync=False,
                                   reason="post-AR math after current group")
                gmean = grT[:, 0, :]
                gvar = stp.tile([128, HOT], f32, tag="gvar", name="gvar")
                veps = stp.tile([128, HOT], f32, tag="veps", name="veps")
                nc.vector.tensor_mul(gvar[:], gmean, gmean)
                nc.vector.tensor_sub(gvar[:], grT[:, 1, :], gvar[:])
                nc.vector.tensor_scalar_add(veps[:], gvar[:], BN_EPS)
                sq = stp.tile([128, HOT], f32, tag="sq", name="sq")
                nc.scalar.sqrt(sq[:], veps[:])
                r = stp.tile([128, HOT], f32, tag="r", name="rstd")
                nc.vector.reciprocal(r[:], sq[:])
                t2 = stp.tile([128, HOT], f32, tag="t2", name="t2")
                for _ in range(2):  # Newton: r <- r * (1.5 - 0.5 * veps * r^2)
                    nc.vector.tensor_mul(t2[:], veps[:], r[:])
                    nc.vector.tensor_mul(t2[:], t2[:], r[:])
                    nc.vector.tensor_scalar(t2[:], t2[:], -0.5, 1.5,
                                            op0=mybir.AluOpType.mult,
                                            op1=mybir.AluOpType.add)
                    nc.vector.tensor_mul(r[:], r[:], t2[:])
                nc.vector.tensor_mul(scal[:, osl], gbt[:, 0, osl], r[:])
                nc.vector.tensor_mul(t2[:], gmean, scal[:, osl])
                nc.vector.tensor_sub(nbias[:, osl], gbt[:, 1, osl], t2[:])

            def norm_group(q):
                for pos in range(GO[q], GO[q] + GS[q]):
                    ob = outp.tile([128, Bc], f16, tag="ob", name="ob")
                    nc.vector.tensor_scalar(
                        ob[:],
                        yTt[:, pos, :],
                        scal[:, pos : pos + 1],
                        nbias[:, pos : pos + 1],
                        op0=mybir.AluOpType.mult,
                        op1=mybir.AluOpType.add,
                    )
                    eng = nc.scalar if pos % 2 else nc.sync
                    eng.dma_start(out=yt[ts(order[pos], 128), :], in_=ob[:])

            # post/norm anchors: group q resolves once its collective has
            # certainly completed; groups 3+4 both resolve at the last
            # position (group 3's normalize fills the final collective's
            # latency window).
            anchor_ot = {13: [0], 18: [1], 25: [2], 29: [3], OT - 1: [4]}

            aggrs = {}
            for pos in range(OT):
                if pos == 0:
                    aggrs[1] = mm_tile(0, head_tiles[0][0],
                                       interleave_with=(1, head_tiles[1][0]))
                    continue
                if pos == 1:
                    continue
                if pos < HEAD:
                    aggrs[pos] = mm_tile(pos, head_tiles[pos][0])
                else:
                    aggrs[pos] = mm_tile(pos, w_load_fp8(order[pos]))
                q = group_end.get(pos)
                if q is not None:
                    stats_pre(q)
                for pq in anchor_ot.get(pos, []):
                    stats_post(pq, anchor=aggrs[pos])
                    norm_group(pq)

    nc.finalize()
    return nc


def shard_inputs(x, w, gamma, beta, n_cores=N_CORES):
    B, IN = x.shape
    OUT = w.shape[0]
    Bc = B // n_cores
    KT, OT = IN // 128, OUT // 128
    OSL = OT // n_cores
    order = proc_order(OT, n_cores, HEAD)
    w4 = np.ascontiguousarray(
        w.reshape(OT, 128, KT, 128).transpose(0, 3, 2, 1)
    )
    whead = np.ascontiguousarray(w4[:HEAD])
    gperm = gamma.reshape(OT, 128)[order].T       # [128, OT] pos-indexed
    bperm = beta.reshape(OT, 128)[order].T
    gbp = np.ascontiguousarray(np.stack([gperm, bperm], axis=1))
    in_maps = []
    for c in range(n_cores):
        xt = np.ascontiguousarray(x[c * Bc : (c + 1) * Bc].T)
        wslc = np.ascontiguousarray(w4[c * OSL : (c + 1) * OSL])
        in_maps.append(
            {"xt": xt, "whead": whead, "wsl": wslc, "gb": gbp}
        )
    return in_maps


_NC_CACHE = {}


def kernel(x, w, gamma, beta):
    x = np.asarray(x)
    w = np.asarray(w)
    gamma = np.asarray(gamma)
    beta = np.asarray(beta)
    B, IN = x.shape
    OUT = w.shape[0]

    key = (B, IN, OUT)
    if key not in _NC_CACHE:
        _NC_CACHE[key] = build(B, IN, OUT)
    nc = _NC_CACHE[key]

    in_maps = shard_inputs(x, w, gamma, beta)
    res = run_bass_kernel_spmd(nc, in_maps, list(range(N_CORES)))
    out = np.concatenate([r["yt"] for r in res.results], axis=1).T
    return np.ascontiguousarray(out.astype(np.float32))


if __name__ == "__main__":
    rng = np.random.default_rng(0)
    B, IN, OUT = 8192, 4096, 4096
    x = rng.standard_normal((B, IN)).astype(np.float32)
    w = rng.standard_normal((OUT, IN)).astype(np.float32)
    gamma = np.ones(OUT, np.float32)
    beta = np.zeros(OUT, np.float32)
    out = kernel(x, w, gamma, beta)
    print(out.shape, out.dtype)


# revision 16
# speedup vs baseline: 1.0459x; 1.0459x over previous
"""Trainium2 Bass kernel for BinaryLinearUnit:
    y = sign(x) @ sign(w).T ; BatchNorm1d(train) ; * gamma + beta

Strategy: data-parallel over the batch dim across 8 NeuronCores, with
the weight matrix distributed as FP8 sign values through AllGathers
instead of every core reading the full fp32 w from HBM (64MB/core in
the f32-replicated version; that kernel was HBM-bound).  Per core:

  - read only the core's own 1/8 slice of w in fp32 (8MB), sign it to
    fp8 on ACT, and distribute it with FOUR slot AllGathers (one per
    owned output tile, 0.5MB in / 4MB out each) that pipeline through
    the collective queue; a single 16MB gather serializes behind the
    ~25us collective latency floor and its transport, stalling the PE.
  - remote output tiles are processed slot-major (4,8,..,28 then
    5,9,..,29, ...) so tile groups become ready in collective order.
  - a small fp32 "head" (output tiles 0..3, replicated to all cores)
    is signed locally so the PE has matmul work while the collectives
    are in flight; ot 0-1 head matmuls are emitted kp-interleaved so
    8 psum chains trickle along with the arriving x stream.
  - x signs are first-class on the ACT queue: head/slice signs are
    spliced between x-tile signs only at points where their own loads
    have certainly landed, so the in-order ACT queue never stalls the
    x->sign->matmul chain on the w path.
  - y.T is computed with fp8 DoubleRow matmuls (sign values exact in
    fp8e4m3, PSUM accumulates fp32; y is an even integer <= 4096,
    exact in f16).
  - sync-BN: per-core [mean/8, E[y^2]/8] AllGathers in 5 groups of
    the processing order sized [7,7,7,7,4] tiles, so the 4th group's
    collective completes before the last matmul and only the final
    small collective plus a short normalize remains in the tail
    (group 3's normalize fills the final collective's latency).
  - normalized output is stored as f16 and widened to f32 on the
    host, halving store traffic.

Indexing: all standing per-tile state (mvT/yTt/scal/nbias/gbt) is
addressed by PROCESSING POSITION; only the yt store offset and the
gathered-w source use the real channel tile.  The host packs gamma/
beta in processing order and un-permutes nothing (yt rows are stored
at their real offsets).

Host side only reshapes/transposes/casts.
"""

import numpy as np

import concourse.bass as bass
import concourse.mybir as mybir
import concourse.tile as tile
from concourse import bacc
from concourse.bass import ts
from concourse.bass_utils import run_bass_kernel_spmd
from concourse.tile_rust import add_dep_helper

N_CORES = 8
BN_EPS = 1e-5
HEAD = 4                      # leading output tiles replicated in f32

f32 = mybir.dt.float32
f16 = mybir.dt.float16
fp8 = mybir.dt.float8e4


def proc_order(OT, n_cores=N_CORES, head=HEAD):
    """head tiles in order, then remote tiles slot-major."""
    OSL = OT // n_cores
    order = list(range(head))
    for j in range(OSL):
        for r in range(n_cores):
            ot = r * OSL + j
            if ot >= head or r > 0:
                if ot not in order:
                    order.append(ot)
    assert len(order) == OT and sorted(order) == list(range(OT))
    return order


def build(B, IN, OUT, n_cores=N_CORES):
    """Build the per-core SPMD module. Shapes: x [B, IN], w [OUT, IN]."""
    Bc = B // n_cores          # batch rows per core
    KT = IN // 128             # k tiles (contraction)
    OT = OUT // 128            # output-feature tiles
    NB = min(512, Bc)          # matmul free dim / psum bank width
    BT = Bc // NB              # b tiles per core
    KP = KT // 2               # DoubleRow k steps
    OSL = OT // n_cores        # output tiles owned per core
    order = proc_order(OT, n_cores, HEAD)

    # BN stats groups over processing positions.
    if OT == 32:
        GS = [7, 7, 7, 5, 6]
    else:
        GS = [OT - OT // 2, OT // 2]
    NSPLIT = len(GS)
    GO = [sum(GS[:q]) for q in range(NSPLIT)]
    group_end = {GO[q] + GS[q] - 1: q for q in range(NSPLIT)}

    nc = bacc.Bacc("TRN2", target_bir_lowering=False, debug=False,
                   num_devices=n_cores)

    # Per-core external I/O (host pre-transposed, K-major):
    #   xt[k, b] = x[core*Bc + b, k]
    #   whead[j, p, ks, o] = w[j*128 + o, ks*128 + p]          (ots 0..HEAD-1)
    #   wsl[j, p, ks, o]   = w[(core*OSL+j)*128 + o, ks*128+p] (own slice)
    #   gb[p, 0, pos] = gamma[order[pos]*128+p] (processing order)
    #   yt[o, b] = out[core*Bc + b, o]  (f16, widened on host)
    xt = nc.dram_tensor("xt", [IN, Bc], f32, kind="ExternalInput")
    whead = nc.dram_tensor("whead", [HEAD, 128, KT, 128], f32,
                           kind="ExternalInput")
    wsl = nc.dram_tensor("wsl", [OSL, 128, KT, 128], f32,
                         kind="ExternalInput")
    gb = nc.dram_tensor("gb", [128, 2, OT], f32, kind="ExternalInput")
    yt = nc.dram_tensor("yt", [OUT, Bc], f16, kind="ExternalOutput")

    # w sign-sharing collective buffers (fp8), one gather per slot.
    wcin = nc.dram_tensor("wcin", [OSL, 128, KT, 128], fp8)
    wcout = [
        nc.dram_tensor(f"wcout{j}", [n_cores, 128, KT, 128], fp8,
                       addr_space="Shared")
        for j in range(OSL)
    ]

    # Stats collective bounce buffers per group: [mean/8, E[y^2]/8].
    ccin = [
        nc.dram_tensor(f"ccin{q}", [128, 2 * GS[q]], f32) for q in range(NSPLIT)
    ]
    ccout = [
        nc.dram_tensor(
            f"ccout{q}", [n_cores * 128, 2 * GS[q]], f32, addr_space="Shared"
        )
        for q in range(NSPLIT)
    ]

    with tile.TileContext(nc) as tc:
        with (
            tc.tile_pool(name="big", bufs=1) as big,
            tc.tile_pool(name="xs", bufs=3) as xsp,
            tc.tile_pool(name="ws", bufs=2) as wsp,
            tc.tile_pool(name="wg", bufs=3) as wgp,
            tc.tile_pool(name="sw", bufs=6) as swp,
            tc.tile_pool(name="wc", bufs=4) as wcp,
            tc.tile_pool(name="ps", bufs=4, space="PSUM") as psp,
            tc.tile_pool(name="st", bufs=2) as stp,
            tc.tile_pool(name="outp", bufs=4) as outp,
        ):
            # Standing tensors (pos-indexed except where noted)
            sxT = big.tile([128, KT, Bc], fp8)          # sign(x).T, K-major
            yTt = big.tile([128, OT, Bc], f16)          # y.T
            mvT = big.tile([128, 2, OT], f32)           # per-core [mean, var]
            gbt = big.tile([128, 2, OT], f32)           # [gamma; beta]
            scal = big.tile([128, OT], f32)             # gamma * rstd
            nbias = big.tile([128, OT], f32)            # beta - mean * scal
            grTs = [None] * NSPLIT                      # global stats tiles

            hk = KT // 2

            # --- head path: ot 0-1 fp32 loads go first on the ACT queue
            # (their signs are spliced into the x stream); ots 2..HEAD-1
            # are loaded+signed after the x loop so the collective input
            # stores are not queued behind them.
            head_tiles = {}     # j -> (swt_fp8, [(dst_slice, src_tile), ...])

            def head_load(j):
                swt = swp.tile([128, KT, 128], fp8, tag="swt", name="swt")
                halves = []
                for h in range(2):
                    wst = wsp.tile([128, hk, 128], f32, tag="wst", name="wst")
                    nc.scalar.dma_start(
                        out=wst[:], in_=whead[j, :, h * hk : (h + 1) * hk, :]
                    )
                    halves.append((swt[:, h * hk : (h + 1) * hk, :], wst))
                head_tiles[j] = (swt, halves)

            head_load(0)
            head_load(1)

            # --- own-slice path (fp32 loads queue behind the head on ACT)
            wcs_tiles = []
            wsl_halves = []
            for j in range(OSL):
                wcs = wcp.tile([128, KT, 128], fp8, tag="wcs", name="wcs")
                for h in range(2):
                    wst = wgp.tile([128, hk, 128], f32, tag="wsl", name="wsl")
                    nc.scalar.dma_start(
                        out=wst[:], in_=wsl[j, :, h * hk : (h + 1) * hk, :]
                    )
                    wsl_halves.append((wcs[:, h * hk : (h + 1) * hk, :], wst))
                wcs_tiles.append(wcs)

            # --- x sign stream: x signs are first-class; head/slice signs
            # and wcin stores + slot collectives are spliced at points
            # where their loads/signs have certainly completed.
            head_sign_after = {2: 0, 3: 1, 6: 2, 8: 3}
            wsl_sign_after = {10: 0, 12: 1, 14: 2, 16: 3, 18: 4, 20: 5,
                              22: 6, 24: 7}
            wcin_after = {13: 0, 17: 1, 21: 2, 25: 3}

            def splice(ks):
                h = head_sign_after.get(ks)
                if h is not None:
                    dst, src = head_tiles[h // 2][1][h % 2]
                    nc.scalar.sign(dst, src[:])
                h = wsl_sign_after.get(ks)
                if h is not None:
                    dst, src = wsl_halves[h]
                    nc.scalar.sign(dst, src[:])
                j = wcin_after.get(ks)
                if j is not None:
                    nc.scalar.dma_start(out=wcin[j], in_=wcs_tiles[j][:])
                    nc.gpsimd.collective_compute(
                        "AllGather",
                        mybir.AluOpType.bypass,
                        replica_groups=[list(range(n_cores))],
                        ins=[wcin[j]],
                        outs=[wcout[j][:]],
                    )

            for ks in range(KT):
                xst = xsp.tile([128, Bc], f32, tag="xst", name="xst")
                nc.sync.dma_start(out=xst[:], in_=xt[ts(ks, 128), :])
                nc.scalar.sign(sxT[:, ks, :], xst[:])
                splice(ks)

            nc.sync.dma_start(out=gbt[:], in_=gb[:])

            # remaining head tiles: loads queue on ACT behind the wcin
            # stores; signs pace themselves behind the loads.
            for j in range(2, HEAD):
                head_load(j)
                for dst, src_t in head_tiles[j][1]:
                    nc.scalar.sign(dst, src_t[:])

            def w_load_fp8(ot):
                swt = swp.tile([128, KT, 128], fp8, tag="swt", name="swt")
                nc.sync.dma_start(out=swt[:], in_=wcout[ot % OSL][ot // OSL])
                return swt

            def mm_tile(pos, swt, interleave_with=None):
                """Matmuls for one (or two kp-interleaved) positions."""
                tiles = [(pos, swt)]
                if interleave_with is not None:
                    tiles.append(interleave_with)
                psums = {
                    (p, bt): psp.tile([128, NB], f32, tag=f"ps{bt}",
                                      name=f"psum{bt}")
                    for p, _ in tiles
                    for bt in range(BT)
                }
                for kp in range(KP):
                    for p, sw in tiles:
                        for bt in range(BT):
                            nc.tensor.matmul(
                                psums[p, bt][:],
                                lhsT=sw[:, 2 * kp : 2 * kp + 2, :],
                                rhs=sxT[:, 2 * kp : 2 * kp + 2, ts(bt, NB)],
                                start=(kp == 0),
                                stop=(kp == KP - 1),
                                perf_mode=mybir.MatmulPerfMode.DoubleRow,
                            )
                aggr = None
                for p, _ in tiles:
                    st6 = stp.tile([128, BT, 6], f32, tag="st6", name="st6",
                                   bufs=4)
                    for bt in range(BT):
                        nc.vector.bn_stats(st6[:, bt, :], psums[p, bt][:])
                        nc.vector.tensor_copy(yTt[:, p, ts(bt, NB)],
                                              psums[p, bt][:])
                    aggr = nc.vector.bn_aggr(mvT[:, :, p], st6[:])
                return aggr

            def stats_pre(q):
                o0, HOT = GO[q], GS[q]
                osl = slice(o0, o0 + HOT)
                arT = stp.tile([128, 2, HOT], f32, tag="arT", name="arT")
                tmp = stp.tile([128, HOT], f32, tag="tmp_ar", name="tmp_ar")
                nc.vector.tensor_scalar_mul(arT[:, 0, :], mvT[:, 0, osl],
                                            1.0 / n_cores)
                nc.vector.tensor_mul(tmp[:], mvT[:, 0, osl], mvT[:, 0, osl])
                nc.vector.tensor_add(tmp[:], tmp[:], mvT[:, 1, osl])
                nc.vector.tensor_scalar_mul(arT[:, 1, :], tmp[:], 1.0 / n_cores)
                nc.sync.dma_start(out=ccin[q][:], in_=arT[:])
                nc.gpsimd.collective_compute(
                    "AllGather",
                    mybir.AluOpType.bypass,
                    replica_groups=[list(range(n_cores))],
                    ins=[ccin[q][:]],
                    outs=[ccout[q][:]],
                )
                grA = big.tile([128, n_cores, 2, HOT], f32, name=f"grA{q}")  # noqa
                nc.gpsimd.dma_start(
                    out=grA[:],
                    in_=ccout[q][:].rearrange("(r p) j -> p r j", p=128),
                )
                grTs[q] = grA

            def stats_post(q, anchor=None):
                o0, HOT = GO[q], GS[q]
                osl = slice(o0, o0 + HOT)
                grA = grTs[q]
                grT = stp.tile([128, 2, HOT], f32, tag="grT", name="grT")
                first = nc.vector.tensor_reduce(
                    grT[:],
                    grA[:].rearrange("p r two h -> p two h r"),
                    axis=mybir.AxisListType.X,
                    op=mybir.AluOpType.add,
                )
                if anchor is not None:
                    add_dep_helper(first.ins, anchor.ins, sync=False,
                                   reason="post-AR math after current group")
                gmean = grT[:, 0, :]
                gvar = stp.tile([128, HOT], f32, tag="gvar", name="gvar")
                veps = stp.tile([128, HOT], f32, tag="veps", name="veps")
                nc.vector.tensor_mul(gvar[:], gmean, gmean)
                nc.vector.tensor_sub(gvar[:], grT[:, 1, :], gvar[:])
                nc.vector.tensor_scalar_add(veps[:], gvar[:], BN_EPS)
                sq = stp.tile([128, HOT], f32, tag="sq", name="sq")
                nc.scalar.sqrt(sq[:], veps[:])
                r = stp.tile([128, HOT], f32, tag="r", name="rstd")
                nc.vector.reciprocal(r[:], sq[:])
                t2 = stp.tile([128, HOT], f32, tag="t2", name="t2")
                for _ in range(2):  # Newton: r <- r * (1.5 - 0.5 * veps * r^2)
                    nc.vector.tensor_mul(t2[:], veps[:], r[:])
                    nc.vector.tensor_mul(t2[:], t2[:], r[:])
                    nc.vector.tensor_scalar(t2[:], t2[:], -0.5, 1.5,
                                            op0=mybir.AluOpType.mult,
                                            op1=mybir.AluOpType.add)
                    nc.vector.tensor_mul(r[:], r[:], t2[:])
                nc.vector.tensor_mul(scal[:, osl], gbt[:, 0, osl], r[:])
                nc.vector.tensor_mul(t2[:], gmean, scal[:, osl])
                nc.vector.tensor_sub(nbias[:, osl], gbt[:, 1, osl], t2[:])

            def norm_group(q):
                for pos in range(GO[q], GO[q] + GS[q]):
                    ob = outp.tile([128, Bc], f16, tag="ob", name="ob")
                    nc.vector.tensor_scalar(
                        ob[:],
                        yTt[:, pos, :],
                        scal[:, pos : pos + 1],
                        nbias[:, pos : pos + 1],
                        op0=mybir.AluOpType.mult,
                        op1=mybir.AluOpType.add,
                    )
                    eng = nc.scalar if pos % 2 else nc.sync
                    eng.dma_start(out=yt[ts(order[pos], 128), :], in_=ob[:])

            # post/norm anchors: group q resolves once its collective has
            # certainly completed; groups 3+4 both resolve at the last
            # position (group 3's normalize fills the final collective's
            # latency window).
            anchor_ot = {13: [0], 18: [1], 25: [2], 28: [3], OT - 1: [4]}

            aggrs = {}
            for pos in range(OT):
                if pos == 0:
                    aggrs[1] = mm_tile(0, head_tiles[0][0],
                                       interleave_with=(1, head_tiles[1][0]))
                    continue
                if pos == 1:
                    continue
                if pos < HEAD:
                    aggrs[pos] = mm_tile(pos, head_tiles[pos][0])
                else:
                    aggrs[pos] = mm_tile(pos, w_load_fp8(order[pos]))
                q = group_end.get(pos)
                if q is not None:
                    stats_pre(q)
                for pq in anchor_ot.get(pos, []):
                    stats_post(pq, anchor=aggrs[pos])
                    norm_group(pq)

    nc.finalize()
    return nc


def shard_inputs(x, w, gamma, beta, n_cores=N_CORES):
    B, IN = x.shape
    OUT = w.shape[0]
    Bc = B // n_cores
    KT, OT = IN // 128, OUT // 128
    OSL = OT // n_cores
    order = proc_order(OT, n_cores, HEAD)
    w4 = np.ascontiguousarray(
        w.reshape(OT, 128, KT, 128).transpose(0, 3, 2, 1)
    )
    whead = np.ascontiguousarray(w4[:HEAD])
    gperm = gamma.reshape(OT, 128)[order].T       # [128, OT] pos-indexed
    bperm = beta.reshape(OT, 128)[order].T
    gbp = np.ascontiguousarray(np.stack([gperm, bperm], axis=1))
    in_maps = []
    for c in range(n_cores):
        xt = np.ascontiguousarray(x[c * Bc : (c + 1) * Bc].T)
        wslc = np.ascontiguousarray(w4[c * OSL : (c + 1) * OSL])
        in_maps.append(
            {"xt": xt, "whead": whead, "wsl": wslc, "gb": gbp}
        )
    return in_maps


_NC_CACHE = {}


def kernel(x, w, gamma, beta):
    x = np.asarray(x)
    w = np.asarray(w)
    gamma = np.asarray(gamma)
    beta = np.asarray(beta)
    B, IN = x.shape
    OUT = w.shape[0]

    key = (B, IN, OUT)
    if key not in _NC_CACHE:
        _NC_CACHE[key] = build(B, IN, OUT)
    nc = _NC_CACHE[key]

    in_maps = shard_inputs(x, w, gamma, beta)
    res = run_bass_kernel_spmd(nc, in_maps, list(range(N_CORES)))
    out = np.concatenate([r["yt"] for r in res.results], axis=1).T
    return np.ascontiguousarray(out.astype(np.float32))


if __name__ == "__main__":
    rng = np.random.default_rng(0)
    B, IN, OUT = 8192, 4096, 4096
    x = rng.standard_normal((B, IN)).astype(np.float32)
    w = rng.standard_normal((OUT, IN)).astype(np.float32)
    gamma = np.ones(OUT, np.float32)
    beta = np.zeros(OUT, np.float32)
    out = kernel(x, w, gamma, beta)
    print(out.shape, out.dtype)
